# revision 44
# baseline (speedup 1.0000x reference)
"""Trainium2 Bass kernel for nn_AttentionComponent_15960098472670.

Reference computation (fp32):
  q = x @ A                      [b, s, 128]
  k = x @ Bmat.T                 [b, s, 128]
  scores = (q*mask) @ k.T / 1024 [b, sq, sk], causal-masked
  patt = softmax(scores)
  out = (patt @ x) @ ov          [b, s, 1024]

Sharding: 8 cores = 4 batches x 2 interleaved query-chunk pairs.
Core (b, h) owns 512-row query chunks {h, h+2} of batch b. With causal
attention, chunk cq only needs key tiles 0..4*(cq+1)-1; padding the two
positions to [8, 16] key-tiles makes the instruction stream identical on
every core (SPMD) while skipping ~25% of the attention FLOPs. Validity
inside the padded range is data: a host-built 0/1 matrix multiplies the
exp'd scores (exact zeros, so softmax denominators stay exact).

On-device layout ("transposed" pipeline, bf16 matmuls / fp32 accumulate):
  qT[c, sq]   = A.T @ xT        (A host-prescaled by 1/1024)
  kT[c, sk]   = Bmat @ xT
  qmT         = qT * maskT
  sT[sk, sq]  = kT-tile.T @ qmT          (PE, per sk-tile of 128)
  PT          = exp(sT) * causal01       (ACT exp psum->sbuf bf16, DVE mul)
  den[1, sq]  = ones.T @ PT              (PE, accumulated over sk-tiles)
  rbc[sq all partitions] = ones1.T @ (1/den)   (rank-1 fp32r broadcast mm)
  zT[d, sq]   = x-tile.T @ PT            (PE, accumulated over sk)
  zb          = zT * rbc                 (DVE psum->sbuf bf16, normalized)
  out[sq, e]  = zb-tile.T @ ov           (PE, accumulated over d; fp32 out)

Scores are tiny (std ~0.006) so exp needs no max-subtraction.
"""

import numpy as np
import ml_dtypes

import concourse.bass as bass
import concourse.mybir as mybir
import concourse.tile as tile
from concourse import bacc
from concourse.bass_utils import run_bass_kernel_spmd

BF16 = mybir.dt.bfloat16
F32 = mybir.dt.float32
F32R = mybir.dt.float32r
FP8 = mybir.dt.float8e4
bfnp = ml_dtypes.bfloat16
fp8np = mybir.dt.np(FP8)

D = 1024      # d_model
C = 128       # channels
S = 2048      # full seq (keys)
SQ = 1024     # queries per core (2 chunks of 512)
P = 128       # partitions
NSK = S // P      # 16 sk tiles
ND = D // P       # 8 d chunks
KPOS = [8, 16]    # padded key-tile count per query-chunk position


def _build_nc():
    nc = bacc.Bacc("TRN2", target_bir_lowering=False, num_devices=8)

    xT_d = nc.dram_tensor("xT", [D, S], FP8, kind="ExternalInput")
    xTq_d = nc.dram_tensor("xTq", [D, SQ], FP8, kind="ExternalInput")
    xn_d = nc.dram_tensor("xn", [S, D], BF16, kind="ExternalInput")
    A_d = nc.dram_tensor("Asc", [P, ND * C], FP8, kind="ExternalInput")
    BT_d = nc.dram_tensor("BT", [P, ND * C], FP8, kind="ExternalInput")
    mT_d = nc.dram_tensor("mT", [C, SQ], FP8, kind="ExternalInput")
    cz_d = nc.dram_tensor("cz", [S, SQ], FP8, kind="ExternalInput")
    ov_d = nc.dram_tensor("ovb", [D, D], BF16, kind="ExternalInput")
    out_d = nc.dram_tensor("out", [SQ, D], F32, kind="ExternalOutput")

    with tile.TileContext(nc) as tc:
        with (
            tc.tile_pool(name="persist", bufs=1) as persist,
            tc.tile_pool(name="pt_pool", bufs=24) as pt_pool,
            tc.tile_pool(name="cz_pool", bufs=16) as cz_pool,
            tc.tile_pool(name="z_pool", bufs=12) as z_pool,
            tc.tile_pool(name="o_pool", bufs=4) as o_pool,
            tc.tile_pool(name="rdn_pool", bufs=2) as rdn_pool,
            tc.tile_pool(name="rb_pool", bufs=2) as rb_pool,
            tc.tile_pool(name="sc_ps", bufs=2, space="PSUM") as sc_ps_pool,
        ):
            # ---- persistent loads (emission order ~= DMA priority) ----
            # one consolidated DMA per tensor/block: per-dma_start fixed cost
            # (~0.6 us HWDGE) dominates with many small transfers
            BT_t = persist.tile([P, ND, C], FP8)
            nc.sync.dma_start(BT_t[:], BT_d.rearrange("p (n c) -> p n c", c=C))
            # xT in key-chunk blocks so kT chunk j computes after ~1 MB each
            xT_t = persist.tile([P, ND, S], FP8)          # 4 MB
            xTq_t = persist.tile([P, ND, SQ], FP8)        # 2 MB

            def xt_block(tile_, dram, j, d0=0, d1=ND):
                nc.sync.dma_start(
                    tile_[:, d0:d1, j * 512:(j + 1) * 512],
                    dram[d0 * P:d1 * P, j * 512:(j + 1) * 512].rearrange(
                        "(n p) s -> p n s", p=P))

            xt_block(xT_t, xT_d, 0, 0, 4)
            xt_block(xT_t, xT_d, 0, 4, 8)
            A_t = persist.tile([P, ND, C], FP8)
            nc.sync.dma_start(A_t[:], A_d.rearrange("p (n c) -> p n c", c=C))

            def xtq_block(j):
                nc.sync.dma_start(
                    xTq_t[:, :, j * 512:(j + 1) * 512],
                    xTq_d[:, j * 512:(j + 1) * 512].rearrange(
                        "(n p) s -> p n s", p=P))

            xtq_block(0)
            xtq_block(1)
            mT_t = persist.tile([P, SQ], FP8)
            nc.sync.dma_start(mT_t[:], mT_d[:, :])
            for j in range(1, 4):
                xt_block(xT_t, xT_d, j)
            xn_t = persist.tile([P, NSK, D], BF16)         # 4 MB, loaded later
            ov_t = persist.tile([P, ND, D], BF16)          # 2 MB, loaded later

            # HAM warmup: junk matmuls during the initial DMA wait so the
            # PE clock-gate opens (1.2 -> 2.4 GHz) before real work arrives
            wu_t = persist.tile([P, 512], BF16)
            nc.vector.memset(wu_t[:], 0.0)
            wu_ps = sc_ps_pool.tile([P, 512], F32, tag="ps", name="wu_ps")
            for _ in range(12):
                nc.tensor.matmul(wu_ps[:], wu_t[:, 0:P], wu_t[:],
                                 start=True, stop=True)

            ones_t = persist.tile([P, 1], BF16)
            nc.vector.memset(ones_t[:], 1.0)
            ones1f_t = persist.tile([1, P], F32)
            nc.vector.memset(ones1f_t[:], 1.0)
            ones1_t = persist.tile([1, P], F32R)
            nc.scalar.copy(ones1_t[:], ones1f_t[:])

            # ---- phase 1: kT [C, S], qmT [C, SQ]; key-chunk-major ----
            kT_t = persist.tile([P, S], FP8)
            qmT_t = persist.tile([P, SQ], FP8)
            with tc.tile_pool(name="acc_ps", bufs=2, space="PSUM") as acc_ps_pool:
                DR = mybir.MatmulPerfMode.DoubleRow

                def k_chunk(j):
                    ps = acc_ps_pool.tile([P, 512], F32, tag="kq", name="kqps")
                    for d in range(ND // 2):
                        nc.tensor.matmul(
                            ps[:], BT_t[:, 2 * d:2 * d + 2, :],
                            xT_t[:, 2 * d:2 * d + 2, j * 512:(j + 1) * 512],
                            start=(d == 0), stop=(d == ND // 2 - 1),
                            perf_mode=DR,
                        )
                    nc.scalar.copy(kT_t[:, j * 512:(j + 1) * 512], ps[:])

                def q_chunk(j):
                    ps = acc_ps_pool.tile([P, 512], F32, tag="kq", name="kqps")
                    for d in range(ND // 2):
                        nc.tensor.matmul(
                            ps[:], A_t[:, 2 * d:2 * d + 2, :],
                            xTq_t[:, 2 * d:2 * d + 2, j * 512:(j + 1) * 512],
                            start=(d == 0), stop=(d == ND // 2 - 1),
                            perf_mode=DR,
                        )
                    nc.scalar.copy(qmT_t[:, j * 512:(j + 1) * 512], ps[:])

                k_chunk(0)
                q_chunk(0)
                q_chunk(1)
                nc.vector.tensor_mul(qmT_t[:], qmT_t[:], mT_t[:])
                k_chunk(1)
                k_chunk(2)
                k_chunk(3)

            # prefetch DMAs, emitted in consumption order: cz p=0 (scores
            # p=0, ~15us), xn rows 0..1023 (z p=0, ~27us), cz p=1 + ov
            # (out p=0 / scores p=1, ~45us), xn rows 1024.. (z p=1, ~57us)
            # position-1 key tiles 0..7 are causally all-valid on every
            # core (keys < 1024 <= any position-1 query), so no cz needed
            czts = {0: [cz_pool.tile([P, 512], FP8, tag="cz", name="czt")
                        for _ in range(8)],
                    1: [None] * 8 + [cz_pool.tile([P, 512], FP8, tag="cz",
                                                  name="czt")
                                     for _ in range(8)]}
            qsl0, qsl1 = slice(0, 512), slice(512, 1024)
            for t in range(8):
                nc.sync.dma_start(czts[0][t][:], cz_d[t * P:(t + 1) * P, qsl0])
            nc.sync.dma_start(
                xn_t[:, 0:ND, :],
                xn_d[0:1024, :].rearrange("(n p) d -> p n d", p=P))
            for t in range(8, 16):
                nc.sync.dma_start(czts[1][t][:], cz_d[t * P:(t + 1) * P, qsl1])
            nc.sync.dma_start(ov_t[:], ov_d.rearrange("(n p) e -> p n e", p=P))
            nc.sync.dma_start(
                xn_t[:, ND:NSK, :],
                xn_d[1024:2048, :].rearrange("(n p) d -> p n d", p=P))

            # ---- phases 2-4 per 512-query chunk position ----
            ctx2 = tc.tile_pool(name="z_ps", bufs=2, space="PSUM")
            z_ps_pool = ctx2.__enter__()
            ctx3 = tc.tile_pool(name="o_ps", bufs=3, space="PSUM")
            o_ps_pool = ctx3.__enter__()
            ctx4 = tc.tile_pool(name="dn_ps", bufs=1, space="PSUM")
            dn_ps_pool = ctx4.__enter__()
            bc_ps_pool = dn_ps_pool  # dn released before bc alloc; share bank
            def score_tile(p, t):
                qsl = slice(p * 512, (p + 1) * 512)
                ps = sc_ps_pool.tile([P, 512], F32, name="ps")
                nc.tensor.matmul(
                    ps[:], kT_t[:, t * P:(t + 1) * P], qmT_t[:, qsl],
                    start=True, stop=True,
                )
                pt = pt_pool.tile([P, 512], BF16, tag="pt", name="pt")
                nc.scalar.activation(pt[:], ps[:],
                                     mybir.ActivationFunctionType.Exp,
                                     scale=1.0 / float(D))
                if czts[p][t] is not None:
                    nc.vector.tensor_mul(pt[:], pt[:], czts[p][t][:])
                return pt

            def dn_block(p, pts):
                dn_ps = dn_ps_pool.tile([1, 512], F32, tag="dnbc", name="dn_ps")
                for t in range(KPOS[p]):
                    nc.tensor.matmul(dn_ps[:], ones_t[:], pts[t][:],
                                     start=(t == 0), stop=(t == KPOS[p] - 1))
                dcp = rdn_pool.tile([1, 512], F32R, name="dcp")
                nc.scalar.copy(dcp[:], dn_ps[:])
                return dcp

            def z_block(p, pts, dcp, after_group=None):
                # zT [d, sq-chunk] = sum_t xn[t].T @ PT[t], normalized by
                # 1/den via a rank-1 broadcast matmul + DVE reciprocal,
                # emitted after the d=0 group so PE never waits on DVE.
                K = KPOS[p]
                zbs = []
                rb = rb_pool.tile([P, 512], F32, name="rb")
                for d in range(ND):
                    z_ps = z_ps_pool.tile([P, 512], F32, name="z_ps")
                    for t in range(K):
                        nc.tensor.matmul(
                            z_ps[:], xn_t[:, t, d * P:(d + 1) * P], pts[t][:],
                            start=(t == 0), stop=(t == K - 1),
                        )
                    if d == 0:
                        bc_ps = bc_ps_pool.tile([P, 512], F32, tag="dnbc",
                                                name="bc_ps")
                        nc.tensor.matmul(bc_ps[:], ones1_t[:], dcp[:],
                                         start=True, stop=True)
                        nc.vector.reciprocal(rb[:], bc_ps[:])
                    if after_group is not None:
                        after_group(d)
                    zb = z_pool.tile([P, 512], BF16, tag="zb", name="zb")
                    nc.vector.tensor_mul(zb[:], z_ps[:], rb[:])
                    zbs.append(zb)
                return zbs

            def out_block(p, zbs):
                for s in range(4):
                    for e in range(2):
                        o_ps = o_ps_pool.tile([P, 512], F32, name="o_ps")
                        for d in range(ND):
                            nc.tensor.matmul(
                                o_ps[:], zbs[d][:, s * P:(s + 1) * P],
                                ov_t[:, d, e * 512:(e + 1) * 512],
                                start=(d == 0), stop=(d == ND - 1),
                            )
                        ot = o_pool.tile([P, 512], F32, tag="ot", name="ot")
                        nc.scalar.copy(ot[:], o_ps[:])
                        nc.sync.dma_start(
                            out_d[p * 512 + s * P:p * 512 + (s + 1) * P,
                                  e * 512:(e + 1) * 512],
                            ot[:],
                        )

            pts0 = [score_tile(0, t) for t in range(KPOS[0])]
            dcp0 = dn_block(0, pts0)
            # scores-p1 matmuls interleave into the z-p0 groups: their exps
            # (ACT-bound) drain while PE does z work
            pts1 = []

            def emit_sc1(d):
                for t in (2 * d, 2 * d + 1):
                    pts1.append(score_tile(1, t))

            zbs0 = z_block(0, pts0, dcp0, after_group=emit_sc1)
            dcp1 = dn_block(1, pts1)
            out_block(0, zbs0)
            zbs1 = z_block(1, pts1, dcp1)
            out_block(1, zbs1)
            ctx4.__exit__(None, None, None)
            ctx3.__exit__(None, None, None)
            ctx2.__exit__(None, None, None)
    nc.compile()
    return nc


_NC_CACHE = None
_LAST_RESULT = None


def kernel(x, A, Bmat, ov, mask):
    global _NC_CACHE, _LAST_RESULT
    B = x.shape[0]
    assert x.shape == (4, S, D) and mask.shape == (4, S, C)

    if _NC_CACHE is None:
        _NC_CACHE = _build_nc()
    nc = _NC_CACHE

    x32 = np.asarray(x, dtype=np.float32)
    def swz(w):  # [D, C] -> [P, ND*C] matching tile layout [p, n, c]
        return np.ascontiguousarray(
            w.reshape(ND, P, C).transpose(1, 0, 2).reshape(P, ND * C))
    Asc = swz(np.asarray(A, dtype=np.float32)).astype(fp8np)
    BT = swz(np.ascontiguousarray(np.asarray(Bmat, dtype=np.float32).T)).astype(fp8np)
    ovb = np.asarray(ov, dtype=np.float32).astype(bfnp)

    kpos = np.arange(S)[:, None]
    in_maps = []
    qrows_all = []
    for c in range(8):
        b, h = c // 2, c % 2
        chunks = [h, h + 2]
        qrows = np.concatenate(
            [np.arange(cq * 512, (cq + 1) * 512) for cq in chunks])
        qrows_all.append(qrows)
        xb = x32[b]
        xT = np.ascontiguousarray(xb.T).astype(fp8np)           # [D, S]
        xTq = np.ascontiguousarray(xb[qrows].T).astype(fp8np)   # [D, SQ]
        xn = xb.astype(bfnp)                                    # [S, D]
        mT = np.ascontiguousarray(mask[b][qrows].T).astype(fp8np)
        cz = (kpos <= qrows[None, :]).astype(fp8np)             # [S, SQ]
        in_maps.append({
            "xT": xT, "xTq": xTq, "xn": xn, "Asc": Asc, "BT": BT,
            "mT": mT, "cz": cz, "ovb": ovb,
        })

    res = run_bass_kernel_spmd(nc, in_maps, core_ids=list(range(8)))
    _LAST_RESULT = res

    out = np.empty((B, S, D), dtype=np.float32)
    for c in range(8):
        b = c // 2
        out[b, qrows_all[c], :] = res.results[c]["out"]
    return out
